# revision 5
# baseline (speedup 1.0000x reference)
"""Multi-head attention (B=2, T=2048, D=2048, H=16) on 8 trn2 NeuronCores.

Sharding: tensor-parallel over heads. Core c owns heads {2c, 2c+1}:
  - QKV projection for its 2 heads (Q^T/K^T in [dh, t] layout, V in [t, dh]).
  - Attention for its 2 heads x 2 batches, computed as S^T = K^T.T Q^T so that
    softmax probs land in [s, t] layout and feed the AV matmul directly
    (no transposes anywhere). Softmax skips max-subtraction (scores are O(15)
    at most here, exp stays well inside fp32 range); the denominator is
    accumulated on DVE + a ones-matvec on PE, broadcast back over partitions
    with a rank-1 PE matmul.
  - AllToAll redistributes attention outputs from head-sharded [dh, t] blocks
    to row-sharded x^T [D, 512] per core; each core then computes 512 rows of
    the output projection with the full W_out (streamed).
Host assembles the 8 row-shards (each returned transposed) into [B, T, D].

Main matmuls run in float32r (single-pass PE fp32, 4x the fp32 rate); the
softmax denominator path stays exact fp32.
"""

import numpy as np

D = 2048
H = 16
DH = 128
B = 2
T = 2048
NT = B * T            # 4096 flattened rows
NCORES = 8
HPC = H // NCORES     # heads per core = 2
ROWS = NT // NCORES   # output rows per core = 512
NFT = D // 128        # 16 feature tiles
SCALE = float(np.sqrt(np.float32(DH)) / np.sqrt(np.float32(D)))  # 0.25

# main matmul operand dtype: "f32r" (single-pass, 4x faster) or "f32" (exact)
MM_DTYPE = "f32r"

_CACHE = {}


def _build():
    import concourse.bass as bass  # noqa: F401
    import concourse.mybir as mybir
    import concourse.tile as tile
    from concourse import bacc

    f32 = mybir.dt.float32
    fmm = mybir.dt.float32r if MM_DTYPE == "f32r" else f32
    Act = mybir.ActivationFunctionType

    nc = bacc.Bacc("TRN2", target_bir_lowering=False, debug=False,
                   num_devices=NCORES)

    xT = nc.dram_tensor("xT", [D, NT], f32, kind="ExternalInput")
    wqkv = nc.dram_tensor("wqkv", [D, 6, DH], f32, kind="ExternalInput")
    bqkv = nc.dram_tensor("bqkv", [6, DH], f32, kind="ExternalInput")
    wout = nc.dram_tensor("wout", [D, D], f32, kind="ExternalInput")
    bout = nc.dram_tensor("bout", [D], f32, kind="ExternalInput")
    outT = nc.dram_tensor("outT", [D, ROWS], f32, kind="ExternalOutput")

    xT_t = xT.rearrange("(n p) t -> p n t", p=128)        # [128, 16, 4096]
    wqkv_t = wqkv.rearrange("(n p) s d -> p n s d", p=128)  # [128, 16, 6, 128]
    wout_t = wout.rearrange("(n p) f -> p n f", p=128)    # [128, 16, 2048]

    ACH = 256                # phase-A t-chunk width
    NACH = NT // ACH         # 16 chunks
    BCH = 512                # phase-B t-chunk width
    NBCH = T // BCH          # 4 chunks per (head, batch)
    NST = T // 128           # 16 s-tiles per batch

    with tile.TileContext(nc) as tc:
        with tc.tile_pool(name="persist", bufs=1) as persist, \
             tc.tile_pool(name="dram", bufs=1, space="DRAM") as dram:
            a2a_in = dram.tile([NCORES, HPC * DH, ROWS], f32)
            a2a_out = dram.tile([NCORES, HPC * DH, ROWS], f32)

            qt_sb = persist.tile([128, HPC, NT], fmm)     # Q^T per head
            kt_sb = persist.tile([128, HPC, NT], fmm)     # K^T per head
            v_sb = persist.tile([128, B, NST, HPC, DH], fmm)  # V natural
            bqkv_sb = persist.tile([128, 6], f32)
            bv_sb = persist.tile([128, HPC * DH], f32)
            bout_sb = persist.tile([128, NFT], f32)
            ones_sb = persist.tile([128, 128], f32)

            nc.sync.dma_start(out=bqkv_sb, in_=bqkv[:, :].transpose([1, 0]))
            nc.sync.dma_start(out=bv_sb,
                              in_=bqkv[4:6, :].flatten().unsqueeze(0)
                              .to_broadcast([128, HPC * DH]))
            nc.sync.dma_start(out=bout_sb,
                              in_=bout.rearrange("(n p) -> p n", p=128))
            nc.vector.memset(ones_sb, 1.0)

            # ---------------- Phase A: QKV projection ----------------
            with tc.tile_pool(name="phaseA", bufs=1) as pa, \
                 tc.tile_pool(name="xtp", bufs=2) as xtp, \
                 tc.tile_pool(name="psA", bufs=2, space="PSUM") as psA:
                wqkv_sb = pa.tile([128, NFT, 6, DH], fmm)
                nc.gpsimd.dma_start(out=wqkv_sb, in_=wqkv_t)

                for tch in range(NACH):
                    t0 = tch * ACH
                    xt_tile = xtp.tile([128, NFT, ACH], fmm, tag="xt")
                    nc.gpsimd.dma_start(out=xt_tile,
                                        in_=xT_t[:, :, t0:t0 + ACH])
                    for h in range(HPC):
                        ps_q = psA.tile([128, ACH], f32, tag="psq")
                        for ft in range(NFT):
                            nc.tensor.matmul(
                                ps_q,
                                wqkv_sb[:, ft, h, :], xt_tile[:, ft, :],
                                start=(ft == 0), stop=(ft == NFT - 1))
                        nc.scalar.activation(
                            out=qt_sb[:, h, t0:t0 + ACH], in_=ps_q,
                            func=Act.Identity, bias=bqkv_sb[:, h:h + 1])
                        ps_k = psA.tile([128, ACH], f32, tag="psk")
                        for ft in range(NFT):
                            nc.tensor.matmul(
                                ps_k,
                                wqkv_sb[:, ft, 2 + h, :], xt_tile[:, ft, :],
                                start=(ft == 0), stop=(ft == NFT - 1))
                        nc.scalar.activation(
                            out=kt_sb[:, h, t0:t0 + ACH], in_=ps_k,
                            func=Act.Identity, bias=bqkv_sb[:, 2 + h:3 + h])
                    for st in range(ACH // 128):
                        ps_v = psA.tile([128, HPC * DH], f32, tag="psv")
                        for ft in range(NFT):
                            nc.tensor.matmul(
                                ps_v,
                                xt_tile[:, ft, st * 128:(st + 1) * 128],
                                wqkv_sb[:, ft, 4:6, :],
                                start=(ft == 0), stop=(ft == NFT - 1))
                        g = t0 + st * 128
                        b_idx, st_b = g // T, (g % T) // 128
                        for h in range(HPC):
                            nc.vector.tensor_add(
                                v_sb[:, b_idx, st_b, h, :],
                                ps_v[:, h * DH:(h + 1) * DH],
                                bv_sb[:, h * DH:(h + 1) * DH])

            # ---------------- Phase B: attention ----------------
            with tc.tile_pool(name="ptp", bufs=2) as ptp, \
                 tc.tile_pool(name="wkB", bufs=2) as wkB, \
                 tc.tile_pool(name="psS", bufs=2, space="PSUM") as psS, \
                 tc.tile_pool(name="psO", bufs=2, space="PSUM") as psO, \
                 tc.tile_pool(name="psX", bufs=1, space="PSUM") as psX:
                for b in range(B):
                    for h in range(HPC):
                        for tc2 in range(NBCH):
                            t0 = b * T + tc2 * BCH
                            pt = ptp.tile([128, NST, BCH], fmm, tag="pt")
                            for sg in range(NST // 2):
                                ps_s = psS.tile([128, 2, BCH], f32, tag="pss")
                                for si in range(2):
                                    st = sg * 2 + si
                                    nc.tensor.matmul(
                                        ps_s[:, si, :],
                                        kt_sb[:, h,
                                              b * T + st * 128:
                                              b * T + (st + 1) * 128],
                                        qt_sb[:, h, t0:t0 + BCH],
                                        start=True, stop=True)
                                nc.scalar.activation(
                                    out=pt[:, sg * 2:(sg + 1) * 2, :],
                                    in_=ps_s, func=Act.Exp, scale=SCALE)
                            # denominator: sum_s exp (DVE accumulate, exact)
                            ptf = pt.bitcast(f32)
                            den_acc = wkB.tile([128, BCH], f32, tag="den")
                            nc.vector.tensor_add(den_acc, ptf[:, 0, :],
                                                 ptf[:, 1, :])
                            for st in range(2, NST):
                                nc.vector.tensor_add(den_acc, den_acc,
                                                     ptf[:, st, :])
                            ps_den = psX.tile([128, BCH], f32, tag="psden")
                            nc.tensor.matmul(ps_den[0:1, :], ones_sb[:, 0:1],
                                             den_acc, start=True, stop=True)
                            den_sb = wkB.tile([128, BCH], f32, tag="densb")
                            nc.vector.tensor_copy(den_sb[0:1, :],
                                                  ps_den[0:1, :])
                            ps_rb = psX.tile([128, BCH], f32, tag="psrb")
                            nc.tensor.matmul(ps_rb, ones_sb[0:1, :],
                                             den_sb[0:1, :],
                                             start=True, stop=True)
                            rb_sb = wkB.tile([128, BCH], f32, tag="rb")
                            nc.vector.reciprocal(rb_sb, ps_rb)
                            # AV
                            ps_o = psO.tile([128, BCH], f32, tag="pso")
                            for st in range(NST):
                                nc.tensor.matmul(
                                    ps_o,
                                    v_sb[:, b, st, h, :], pt[:, st, :],
                                    start=(st == 0), stop=(st == NST - 1))
                            obuf = wkB.tile([128, BCH], f32, tag="obuf")
                            nc.vector.tensor_mul(obuf, ps_o, rb_sb)
                            j = b * NBCH + tc2
                            nc.sync.dma_start(
                                out=a2a_in[j, h * DH:(h + 1) * DH, :],
                                in_=obuf)

            # ---------------- AllToAll ----------------
            nc.gpsimd.collective_compute(
                "AllToAll", mybir.AluOpType.bypass,
                replica_groups=[list(range(NCORES))],
                ins=[a2a_in.opt()], outs=[a2a_out.opt()])

            # ---------------- Phase C: output projection ----------------
            with tc.tile_pool(name="phaseC", bufs=1) as pc, \
                 tc.tile_pool(name="woutp", bufs=2) as woutp, \
                 tc.tile_pool(name="outp", bufs=3) as outp, \
                 tc.tile_pool(name="psC", bufs=2, space="PSUM") as psC:
                xr_sb = pc.tile([128, NFT, ROWS], fmm)
                nc.gpsimd.dma_start(
                    out=xr_sb,
                    in_=a2a_out.rearrange("c (n p) r -> p (c n) r", p=128))
                for fch in range(4):
                    wout_tile = woutp.tile([128, NFT, 512], fmm, tag="wo")
                    nc.gpsimd.dma_start(
                        out=wout_tile,
                        in_=wout_t[:, :, fch * 512:(fch + 1) * 512])
                    for fsub in range(4):
                        ps_c = psC.tile([128, ROWS], f32, tag="psc")
                        for kt_i in range(NFT):
                            nc.tensor.matmul(
                                ps_c,
                                wout_tile[:, kt_i,
                                          fsub * 128:(fsub + 1) * 128],
                                xr_sb[:, kt_i, :],
                                start=(kt_i == 0), stop=(kt_i == NFT - 1))
                        fti = fch * 4 + fsub
                        out_sb = outp.tile([128, ROWS], f32, tag="ob")
                        nc.scalar.activation(
                            out=out_sb, in_=ps_c, func=Act.Identity,
                            bias=bout_sb[:, fti:fti + 1])
                        nc.sync.dma_start(
                            out=outT[fti * 128:(fti + 1) * 128, :],
                            in_=out_sb)

    nc.compile()
    return nc


def get_nc():
    key = ("nc", MM_DTYPE)
    if key not in _CACHE:
        _CACHE[key] = _build()
    return _CACHE[key]


def make_in_maps(query, W_qkv, b_qkv, W_out, b_out):
    query = np.asarray(query, dtype=np.float32)
    W_qkv = np.asarray(W_qkv, dtype=np.float32)
    b_qkv = np.asarray(b_qkv, dtype=np.float32)
    W_out = np.ascontiguousarray(np.asarray(W_out, dtype=np.float32))
    b_out = np.ascontiguousarray(np.asarray(b_out, dtype=np.float32))

    x = query.reshape(NT, D)
    xT = np.ascontiguousarray(x.T)                      # [D, NT]

    in_maps = []
    for c in range(NCORES):
        h0, h1 = HPC * c, HPC * c + 1
        cols, bcols = [], []
        for blk in range(3):  # q, k, v blocks of W_qkv
            for h in (h0, h1):
                sl = slice(blk * D + h * DH, blk * D + (h + 1) * DH)
                cols.append(W_qkv[:, sl])
                bcols.append(b_qkv[sl])
        in_maps.append({
            "xT": xT,
            "wqkv": np.ascontiguousarray(np.stack(cols, axis=1)),
            "bqkv": np.ascontiguousarray(np.stack(bcols, axis=0)),
            "wout": W_out,
            "bout": b_out,
        })
    return in_maps


def kernel(query, key, value, W_qkv, b_qkv, W_out, b_out):
    from concourse.bass_utils import run_bass_kernel_spmd

    nc = get_nc()
    in_maps = make_in_maps(query, W_qkv, b_qkv, W_out, b_out)
    res = run_bass_kernel_spmd(nc, in_maps, list(range(NCORES)))
    out = np.empty((NT, D), dtype=np.float32)
    for c in range(NCORES):
        out[c * ROWS:(c + 1) * ROWS, :] = res.results[c]["outT"].T
    return out.reshape(B, T, D)


# revision 7
# speedup vs baseline: 1.1064x; 1.1064x over previous
"""Multi-head attention (B=2, T=2048, D=2048, H=16) on 8 trn2 NeuronCores.

Sharding: tensor-parallel over heads. Core c owns heads {2c, 2c+1}:
  - QKV projection for its 2 heads (Q^T/K^T in [dh, t] layout, V in [t, dh]).
  - Attention per (head, batch), computed as S^T = K^T.T Q^T so softmax probs
    land in [s, t] layout and feed the AV matmul directly (no transposes).
    Softmax skips max-subtraction (scores are O(15) here; exp stays well
    inside fp32 range). The denominator is accumulated on DVE, partition-
    reduced by a ones-matvec on PE, reciprocated at [1, t] and broadcast back
    over partitions with a rank-1 PE matmul.
  - Two per-head AllToAlls redistribute attention outputs from head-sharded
    [dh, t] blocks to row-sharded x^T [D, 512] per core; the first overlaps
    with the second head's attention. Each core then computes 512 rows of the
    output projection with the full W_out (streamed).
Host assembles the 8 row-shards (each returned transposed) into [B, T, D].

All main matmuls run in float32r (single-pass PE fp32). Inputs are declared
float32r in DRAM so plain HWDGE DMAs (no casting) feed SBUF.
"""

import numpy as np

D = 2048
H = 16
DH = 128
B = 2
T = 2048
NT = B * T            # 4096 flattened rows
NCORES = 8
HPC = H // NCORES     # heads per core = 2
ROWS = NT // NCORES   # output rows per core = 512
NFT = D // 128        # 16 feature tiles
SCALE = float(np.sqrt(np.float32(DH)) / np.sqrt(np.float32(D)))  # 0.25

MM_DTYPE = "f32r"

_CACHE = {}


def _build():
    import concourse.bass as bass  # noqa: F401
    import concourse.mybir as mybir
    import concourse.tile as tile
    from concourse import bacc

    f32 = mybir.dt.float32
    fmm = mybir.dt.float32r if MM_DTYPE == "f32r" else f32
    Act = mybir.ActivationFunctionType

    nc = bacc.Bacc("TRN2", target_bir_lowering=False, debug=False,
                   num_devices=NCORES)

    xT = nc.dram_tensor("xT", [D, NT], fmm, kind="ExternalInput")
    wqkv = nc.dram_tensor("wqkv", [D, 6, DH], fmm, kind="ExternalInput")
    bqkv = nc.dram_tensor("bqkv", [6, DH], f32, kind="ExternalInput")
    wout = nc.dram_tensor("wout", [D, D], fmm, kind="ExternalInput")
    bout = nc.dram_tensor("bout", [D], f32, kind="ExternalInput")
    outT = nc.dram_tensor("outT", [D, ROWS], f32, kind="ExternalOutput")

    xT_t = xT.rearrange("(n p) t -> p n t", p=128)        # [128, 16, 4096]
    wqkv_t = wqkv.rearrange("(n p) s d -> p n s d", p=128)  # [128, 16, 6, 128]
    wout_t = wout.rearrange("(n p) f -> p n f", p=128)    # [128, 16, 2048]

    ACH = 256                # phase-A t-chunk width
    NACH = NT // ACH         # 16 chunks
    BCH = 512                # phase-B t-chunk width
    NBCH = T // BCH          # 4 chunks per (head, batch)
    NST = T // 128           # 16 s-tiles per batch

    with tile.TileContext(nc) as tc:
        with tc.tile_pool(name="persist", bufs=1) as persist, \
             tc.tile_pool(name="dram", bufs=1, space="DRAM") as dram:
            a2a_in = [dram.tile([NCORES, DH, ROWS], fmm, name=f"a2a_in{h}")
                      for h in range(HPC)]
            a2a_out = [dram.tile([NCORES, DH, ROWS], fmm, name=f"a2a_out{h}")
                       for h in range(HPC)]

            qt_sb = persist.tile([128, HPC, NT], fmm)     # Q^T per head
            kt_sb = persist.tile([128, HPC, NT], fmm)     # K^T per head
            v_sb = persist.tile([128, B, NST, HPC, DH], fmm)  # V natural
            bqkv_sb = persist.tile([128, 6], f32)
            bv_sb = persist.tile([128, HPC * DH], f32)
            bout_sb = persist.tile([128, NFT], f32)
            ones_sb = persist.tile([128, 128], f32)

            nc.sync.dma_start(out=bqkv_sb, in_=bqkv[:, :].transpose([1, 0]))
            nc.sync.dma_start(out=bv_sb,
                              in_=bqkv[4:6, :].flatten().unsqueeze(0)
                              .to_broadcast([128, HPC * DH]))
            nc.sync.dma_start(out=bout_sb,
                              in_=bout.rearrange("(n p) -> p n", p=128))
            nc.vector.memset(ones_sb, 1.0)

            # ---------------- Phase A: QKV projection ----------------
            with tc.tile_pool(name="phaseA", bufs=1) as pa, \
                 tc.tile_pool(name="xtp", bufs=2) as xtp, \
                 tc.tile_pool(name="psA", bufs=2, space="PSUM") as psA:
                wqkv_sb = pa.tile([128, NFT, 6, DH], fmm)
                # per-f-tile DMAs so the first accumulation chain can start
                # before the whole 6MB lands
                for ft in range(NFT):
                    nc.sync.dma_start(out=wqkv_sb[:, ft, :, :],
                                      in_=wqkv_t[:, ft, :, :])

                for tch in range(NACH):
                    t0 = tch * ACH
                    xt_tile = xtp.tile([128, NFT, ACH], fmm, tag="xt")
                    nc.sync.dma_start(out=xt_tile,
                                      in_=xT_t[:, :, t0:t0 + ACH])
                    for h in range(HPC):
                        ps_q = psA.tile([128, ACH], f32, tag="psq")
                        for ft in range(NFT):
                            nc.tensor.matmul(
                                ps_q,
                                wqkv_sb[:, ft, h, :], xt_tile[:, ft, :],
                                start=(ft == 0), stop=(ft == NFT - 1))
                        nc.scalar.activation(
                            out=qt_sb[:, h, t0:t0 + ACH], in_=ps_q,
                            func=Act.Identity, bias=bqkv_sb[:, h:h + 1])
                        ps_k = psA.tile([128, ACH], f32, tag="psk")
                        for ft in range(NFT):
                            nc.tensor.matmul(
                                ps_k,
                                wqkv_sb[:, ft, 2 + h, :], xt_tile[:, ft, :],
                                start=(ft == 0), stop=(ft == NFT - 1))
                        nc.scalar.activation(
                            out=kt_sb[:, h, t0:t0 + ACH], in_=ps_k,
                            func=Act.Identity, bias=bqkv_sb[:, 2 + h:3 + h])
                    for st in range(ACH // 128):
                        ps_v = psA.tile([128, HPC * DH], f32, tag="psv")
                        for ft in range(NFT):
                            nc.tensor.matmul(
                                ps_v,
                                xt_tile[:, ft, st * 128:(st + 1) * 128],
                                wqkv_sb[:, ft, 4:6, :],
                                start=(ft == 0), stop=(ft == NFT - 1))
                        g = t0 + st * 128
                        b_idx, st_b = g // T, (g % T) // 128
                        for h in range(HPC):
                            nc.vector.tensor_add(
                                v_sb[:, b_idx, st_b, h, :],
                                ps_v[:, h * DH:(h + 1) * DH],
                                bv_sb[:, h * DH:(h + 1) * DH])

            # ---------------- Phase B: attention (h outer for A2A split) ----
            with tc.tile_pool(name="ptp", bufs=2) as ptp, \
                 tc.tile_pool(name="wkB", bufs=2) as wkB, \
                 tc.tile_pool(name="psS", bufs=2, space="PSUM") as psS, \
                 tc.tile_pool(name="psO", bufs=2, space="PSUM") as psO, \
                 tc.tile_pool(name="psX", bufs=1, space="PSUM") as psX:
                for h in range(HPC):
                    for b in range(B):
                        for tc2 in range(NBCH):
                            t0 = b * T + tc2 * BCH
                            pt = ptp.tile([128, NST, BCH], fmm, tag="pt")
                            for sg in range(NST // 2):
                                ps_s = psS.tile([128, 2, BCH], f32, tag="pss")
                                for si in range(2):
                                    st = sg * 2 + si
                                    nc.tensor.matmul(
                                        ps_s[:, si, :],
                                        kt_sb[:, h,
                                              b * T + st * 128:
                                              b * T + (st + 1) * 128],
                                        qt_sb[:, h, t0:t0 + BCH],
                                        start=True, stop=True)
                                nc.scalar.activation(
                                    out=pt[:, sg * 2:(sg + 1) * 2, :],
                                    in_=ps_s, func=Act.Exp, scale=SCALE)
                            # denominator: sum_s exp — DVE tree reduction
                            ptf = pt.bitcast(f32)
                            acc8 = wkB.tile([128, 8, BCH], f32, tag="acc8")
                            nc.vector.tensor_add(acc8, ptf[:, 0:8, :],
                                                 ptf[:, 8:16, :])
                            nc.vector.tensor_add(acc8[:, 0:4, :],
                                                 acc8[:, 0:4, :],
                                                 acc8[:, 4:8, :])
                            nc.vector.tensor_add(acc8[:, 0:2, :],
                                                 acc8[:, 0:2, :],
                                                 acc8[:, 2:4, :])
                            nc.vector.tensor_add(acc8[:, 0, :],
                                                 acc8[:, 0, :],
                                                 acc8[:, 1, :])
                            ps_den = psX.tile([128, BCH], f32, tag="psden")
                            nc.tensor.matmul(ps_den[0:1, :], ones_sb[:, 0:1],
                                             acc8[:, 0, :],
                                             start=True, stop=True)
                            den_sb = wkB.tile([1, BCH], f32, tag="densb")
                            nc.vector.reciprocal(den_sb[0:1, :],
                                                 ps_den[0:1, :])
                            ps_rb = psX.tile([128, BCH], f32, tag="psrb")
                            nc.tensor.matmul(ps_rb, ones_sb[0:1, :],
                                             den_sb[0:1, :],
                                             start=True, stop=True)
                            rb_sb = wkB.tile([128, BCH], f32, tag="rb")
                            nc.vector.tensor_copy(rb_sb, ps_rb)
                            # AV
                            ps_o = psO.tile([128, BCH], f32, tag="pso")
                            for st in range(NST):
                                nc.tensor.matmul(
                                    ps_o,
                                    v_sb[:, b, st, h, :], pt[:, st, :],
                                    start=(st == 0), stop=(st == NST - 1))
                            obuf = wkB.tile([128, BCH], fmm, tag="obuf")
                            nc.vector.tensor_mul(obuf, ps_o, rb_sb)
                            j = b * NBCH + tc2
                            nc.sync.dma_start(out=a2a_in[h][j, :, :],
                                              in_=obuf)
                    # per-head AllToAll; head 0's overlaps head 1's compute
                    nc.gpsimd.collective_compute(
                        "AllToAll", mybir.AluOpType.bypass,
                        replica_groups=[list(range(NCORES))],
                        ins=[a2a_in[h].opt()], outs=[a2a_out[h].opt()])

            # ---------------- Phase C: output projection ----------------
            with tc.tile_pool(name="phaseC", bufs=1) as pc, \
                 tc.tile_pool(name="woutp", bufs=2) as woutp, \
                 tc.tile_pool(name="outp", bufs=3) as outp, \
                 tc.tile_pool(name="psC", bufs=2, space="PSUM") as psC:
                xr_sb = [pc.tile([128, NCORES, ROWS], fmm, name=f"xr{h}")
                         for h in range(HPC)]
                for h in range(HPC):
                    nc.sync.dma_start(
                        out=xr_sb[h],
                        in_=a2a_out[h].rearrange("c p r -> p c r"))
                for fch in range(4):
                    wout_tile = woutp.tile([128, NFT, 512], fmm, tag="wo")
                    nc.sync.dma_start(
                        out=wout_tile,
                        in_=wout_t[:, :, fch * 512:(fch + 1) * 512])
                    for fsub in range(4):
                        ps_c = psC.tile([128, ROWS], f32, tag="psc")
                        for hh in range(HPC):
                            for c in range(NCORES):
                                nc.tensor.matmul(
                                    ps_c,
                                    wout_tile[:, HPC * c + hh,
                                              fsub * 128:(fsub + 1) * 128],
                                    xr_sb[hh][:, c, :],
                                    start=(hh == 0 and c == 0),
                                    stop=(hh == HPC - 1 and c == NCORES - 1))
                        fti = fch * 4 + fsub
                        out_sb = outp.tile([128, ROWS], f32, tag="ob")
                        nc.scalar.activation(
                            out=out_sb, in_=ps_c, func=Act.Identity,
                            bias=bout_sb[:, fti:fti + 1])
                        nc.sync.dma_start(
                            out=outT[fti * 128:(fti + 1) * 128, :],
                            in_=out_sb)

    nc.compile()
    return nc


def get_nc():
    key = ("nc", MM_DTYPE)
    if key not in _CACHE:
        _CACHE[key] = _build()
    return _CACHE[key]


def make_in_maps(query, W_qkv, b_qkv, W_out, b_out):
    query = np.asarray(query, dtype=np.float32)
    W_qkv = np.asarray(W_qkv, dtype=np.float32)
    b_qkv = np.asarray(b_qkv, dtype=np.float32)
    W_out = np.ascontiguousarray(np.asarray(W_out, dtype=np.float32))
    b_out = np.ascontiguousarray(np.asarray(b_out, dtype=np.float32))

    x = query.reshape(NT, D)
    xT = np.ascontiguousarray(x.T)                      # [D, NT]

    in_maps = []
    for c in range(NCORES):
        h0, h1 = HPC * c, HPC * c + 1
        cols, bcols = [], []
        for blk in range(3):  # q, k, v blocks of W_qkv
            for h in (h0, h1):
                sl = slice(blk * D + h * DH, blk * D + (h + 1) * DH)
                cols.append(W_qkv[:, sl])
                bcols.append(b_qkv[sl])
        in_maps.append({
            "xT": xT,
            "wqkv": np.ascontiguousarray(np.stack(cols, axis=1)),
            "bqkv": np.ascontiguousarray(np.stack(bcols, axis=0)),
            "wout": W_out,
            "bout": b_out,
        })
    return in_maps


def kernel(query, key, value, W_qkv, b_qkv, W_out, b_out):
    from concourse.bass_utils import run_bass_kernel_spmd

    nc = get_nc()
    in_maps = make_in_maps(query, W_qkv, b_qkv, W_out, b_out)
    res = run_bass_kernel_spmd(nc, in_maps, list(range(NCORES)))
    out = np.empty((NT, D), dtype=np.float32)
    for c in range(NCORES):
        out[c * ROWS:(c + 1) * ROWS, :] = res.results[c]["outT"].T
    return out.reshape(B, T, D)


# revision 29
# speedup vs baseline: 1.2897x; 1.1657x over previous
"""Multi-head attention (B=2, T=2048, D=2048, H=16) on 8 trn2 NeuronCores.

Sharding: tensor-parallel over heads. Core c owns heads {2c, 2c+1}:
  - QKV projection for its 2 heads (Q^T/K^T in [dh, t] layout, V in [t, dh]).
  - Attention per (head, batch), computed as S^T = K^T.T Q^T so softmax probs
    land in [s, t] layout and feed the AV matmul directly (no transposes).
    Softmax skips max-subtraction (scores are O(15) here; exp stays well
    inside fp32 range). The denominator is accumulated on DVE, partition-
    reduced by a ones-matvec on PE, reciprocated at [1, t] and broadcast back
    over partitions with a rank-1 PE matmul.
  - Two per-head AllToAlls redistribute attention outputs from head-sharded
    [dh, t] blocks to row-sharded x^T [D, 512] per core; the first overlaps
    with the second head's attention. Each core then computes 512 rows of the
    output projection with the full W_out (streamed).
Host assembles the 8 row-shards (each returned transposed) into [B, T, D].

All main matmuls run in float32r (single-pass PE fp32). Inputs are declared
float32r in DRAM so plain HWDGE DMAs (no casting) feed SBUF.
"""

import numpy as np

D = 2048
H = 16
DH = 128
B = 2
T = 2048
NT = B * T            # 4096 flattened rows
NCORES = 8
HPC = H // NCORES     # heads per core = 2
ROWS = NT // NCORES   # output rows per core = 512
NFT = D // 128        # 16 feature tiles
SCALE = float(np.sqrt(np.float32(DH)) / np.sqrt(np.float32(D)))  # 0.25

MM_DTYPE = "f32r"

_CACHE = {}


def _build():
    from contextlib import ExitStack

    import concourse.bass as bass  # noqa: F401
    import concourse.mybir as mybir
    import concourse.tile as tile
    from concourse import bacc

    f32 = mybir.dt.float32
    fmm = mybir.dt.float32r if MM_DTYPE == "f32r" else f32
    bf16 = mybir.dt.bfloat16
    Act = mybir.ActivationFunctionType

    nc = bacc.Bacc("TRN2", target_bir_lowering=False, debug=False,
                   num_devices=NCORES)

    ACH = 256                # phase-A t-chunk width
    NACH = NT // ACH         # 16 chunks

    # inputs are pre-arranged on the host into SBUF-image layouts so every
    # load is a fully contiguous per-partition DMA
    xT = nc.dram_tensor("xT", [128, NACH, NFT, ACH], fmm,
                        kind="ExternalInput")
    wqkv = nc.dram_tensor("wqkv", [128, 6, NFT, DH], fmm,
                          kind="ExternalInput")
    bqkv = nc.dram_tensor("bqkv", [6, DH], f32, kind="ExternalInput")
    wout = nc.dram_tensor("wout", [128, 8, NFT, 256], bf16,
                          kind="ExternalInput")
    bout = nc.dram_tensor("bout", [D], f32, kind="ExternalInput")
    outT = nc.dram_tensor("outT", [D, ROWS], f32, kind="ExternalOutput")
    BCH = 512                # phase-B t-chunk width
    NBCH = T // BCH          # 4 chunks per (head, batch)
    NST = T // 128           # 16 s-tiles per batch

    with tile.TileContext(nc) as tc, ExitStack() as es:
        persist = es.enter_context(tc.tile_pool(name="persist", bufs=1))
        dram = es.enter_context(tc.tile_pool(name="dram", bufs=1,
                                             space="DRAM"))
        a2a_in = [dram.tile([NCORES, DH, ROWS], bf16, name=f"a2a_in{h}")
                  for h in range(HPC)]
        a2a_out = [dram.tile([NCORES, DH, ROWS], bf16, name=f"a2a_out{h}")
                   for h in range(HPC)]

        bqkv_sb = persist.tile([128, 6], f32)
        bv_sb = persist.tile([128, HPC * DH], f32)
        bout_sb = persist.tile([128, NFT], f32)
        ones_sb = persist.tile([128, 128], f32)
        ones_r = persist.tile([128, 128], fmm)

        nc.sync.dma_start(out=bqkv_sb, in_=bqkv[:, :].transpose([1, 0]))
        nc.sync.dma_start(out=bv_sb,
                          in_=bqkv[4:6, :].flatten().unsqueeze(0)
                          .to_broadcast([128, HPC * DH]))
        nc.sync.dma_start(out=bout_sb,
                          in_=bout.rearrange("(n p) -> p n", p=128))
        nc.vector.memset(ones_sb, 1.0)
        nc.vector.tensor_copy(ones_r, ones_sb)

        xr0_sb = persist.tile([128, NCORES, ROWS], bf16)
        mides = ExitStack()
        mid = mides.enter_context(tc.tile_pool(name="mid", bufs=1))
        qt_sb = [mid.tile([128, NT], fmm, name=f"qt{h}") for h in range(HPC)]
        kt_sb = [mid.tile([128, NT], fmm, name=f"kt{h}") for h in range(HPC)]
        v_sb = [mid.tile([128, B, NST, DH], fmm, name=f"v{h}")
                for h in range(HPC)]

        # ---------------- Phase A: QKV projection ----------------
        with tc.tile_pool(name="phaseA", bufs=1) as pa, \
             tc.tile_pool(name="xtp", bufs=2) as xtp, \
             tc.tile_pool(name="psA", bufs=2, space="PSUM") as psA:
            wqkv_sb = pa.tile([128, 6, NFT, DH], fmm)
            # slot-major loads: the first Q chain only needs slot 0 (1MB);
            # xt0 goes first on the scalar queue in quarters so the first
            # accumulation chain starts as soon as slot 0 + quarter 0 land
            # slot 0 (first Q chain) split in halves on sync
            for hlf in range(2):
                nc.sync.dma_start(
                    out=wqkv_sb[:, 0, 8 * hlf:8 * hlf + 8, :],
                    in_=wqkv[:, 0, 8 * hlf:8 * hlf + 8, :])
            nc.sync.dma_start(out=wqkv_sb[:, 2, :, :], in_=wqkv[:, 2, :, :])
            nc.sync.dma_start(out=wqkv_sb[:, 4, :, :], in_=wqkv[:, 4, :, :])

            for tch in range(NACH):
                t0 = tch * ACH
                xt_tile = xtp.tile([128, NFT, ACH], fmm, tag="xt")
                if tch == 0:
                    for q in range(4):
                        nc.scalar.dma_start(
                            out=xt_tile[:, 4 * q:4 * q + 4, :],
                            in_=xT[:, 0, 4 * q:4 * q + 4, :])
                    for s in (1, 3, 5):
                        nc.scalar.dma_start(out=wqkv_sb[:, s, :, :],
                                            in_=wqkv[:, s, :, :])
                else:
                    nc.gpsimd.dma_start(out=xt_tile, in_=xT[:, tch, :, :])
                for h in range(HPC):
                    ps_q = psA.tile([128, ACH], f32, tag="psq")
                    for ft in range(NFT):
                        nc.tensor.matmul(
                            ps_q,
                            wqkv_sb[:, h, ft, :], xt_tile[:, ft, :],
                            start=(ft == 0), stop=(ft == NFT - 1))
                    nc.scalar.activation(
                        out=qt_sb[h][:, t0:t0 + ACH], in_=ps_q,
                        func=Act.Identity, bias=bqkv_sb[:, h:h + 1])
                    ps_k = psA.tile([128, ACH], f32, tag="psk")
                    for ft in range(NFT):
                        nc.tensor.matmul(
                            ps_k,
                            wqkv_sb[:, 2 + h, ft, :], xt_tile[:, ft, :],
                            start=(ft == 0), stop=(ft == NFT - 1))
                    nc.scalar.activation(
                        out=kt_sb[h][:, t0:t0 + ACH], in_=ps_k,
                        func=Act.Identity, bias=bqkv_sb[:, 2 + h:3 + h])
                for st in range(ACH // 128):
                    ps_v = psA.tile([128, HPC * DH], f32, tag="psv")
                    for ft in range(NFT):
                        nc.tensor.matmul(
                            ps_v,
                            xt_tile[:, ft, st * 128:(st + 1) * 128],
                            wqkv_sb[:, 4:6, ft, :],
                            start=(ft == 0), stop=(ft == NFT - 1))
                    g = t0 + st * 128
                    b_idx, st_b = g // T, (g % T) // 128
                    for h in range(HPC):
                        nc.vector.tensor_add(
                            v_sb[h][:, b_idx, st_b, :],
                            ps_v[:, h * DH:(h + 1) * DH],
                            bv_sb[:, h * DH:(h + 1) * DH])

        # ---------------- Phase B: attention (h outer, split A2A) --------
        # Software pipeline: chunk k's AV matmuls interleave with chunk k+1's
        # S matmuls so PE stays busy while ACT works through the exps.
        with tc.tile_pool(name="ptp", bufs=2) as ptp, \
             tc.tile_pool(name="wkB", bufs=2) as wkB, \
             tc.tile_pool(name="psS", bufs=1, space="PSUM") as psS, \
             tc.tile_pool(name="psO", bufs=3, space="PSUM") as psO, \
             tc.tile_pool(name="psX", bufs=1, space="PSUM") as psX:

            def emit_av(pend, st_list):
                h, b, j, pt_p = pend["h"], pend["b"], pend["j"], pend["pt"]
                if pend["ps_o"] is None:
                    ps_o = psO.tile([128, BCH], f32, tag="pso",
                                    name=f"pso{h}_{j}")
                    pend["ps_o"] = ps_o
                for st in st_list:
                    nc.tensor.matmul(
                        pend["ps_o"],
                        v_sb[h][:, b, st, :], pt_p[:, st, :],
                        start=(st == 0), stop=(st == NST - 1))

            def emit_tree_step(pend, step):
                h, j, pt_p = pend["h"], pend["j"], pend["pt"]
                ptf = pt_p.bitcast(f32)
                if step == 0:
                    acc4 = wkB.tile([128, 4, BCH], fmm, tag="acc4", bufs=1,
                                    name=f"acc4_{h}_{j}")
                    pend["acc4"] = acc4
                acc4 = pend["acc4"]
                acc4f = acc4.bitcast(f32)
                if step == 0:
                    nc.vector.tensor_add(acc4, ptf[:, 0:4, :],
                                         ptf[:, 4:8, :])
                elif step == 1:
                    nc.vector.tensor_add(acc4, acc4f, ptf[:, 8:12, :])
                elif step == 2:
                    nc.vector.tensor_add(acc4, acc4f, ptf[:, 12:16, :])
                elif step == 3:
                    nc.vector.tensor_add(acc4[:, 0:2, :], acc4f[:, 0:2, :],
                                         acc4f[:, 2:4, :])
                else:
                    nc.vector.tensor_add(acc4[:, 0, :], acc4f[:, 0, :],
                                         acc4f[:, 1, :])

            def emit_epilogue(pend):
                h, b, j, pt_p = pend["h"], pend["b"], pend["j"], pend["pt"]
                for step in range(pend["tree_step"], 5):
                    emit_tree_step(pend, step)
                acc4 = pend["acc4"]
                ps_den = psX.tile([128, BCH], f32, tag="psx",
                                  name=f"psden{h}_{j}")
                nc.tensor.matmul(ps_den[0:1, :], ones_r[:, 0:1],
                                 acc4.bitcast(fmm)[:, 0, :],
                                 start=True, stop=True)
                den_sb = wkB.tile([1, BCH], f32, tag="densb",
                                  name=f"den{h}_{j}")
                nc.vector.reciprocal_approx_fast(den_sb[0:1, :],
                                                 ps_den[0:1, :])
                den_r = wkB.tile([1, BCH], fmm, tag="denr",
                                 name=f"denr{h}_{j}")
                nc.vector.tensor_copy(den_r[0:1, :], den_sb[0:1, :])
                ps_rb = psX.tile([128, BCH], f32, tag="psx",
                                 name=f"psrb{h}_{j}")
                nc.tensor.matmul(ps_rb, ones_r[0:1, :], den_r[0:1, :],
                                 start=True, stop=True)
                rb_sb = wkB.tile([128, BCH], f32, tag="rb",
                                 name=f"rb{h}_{j}")
                nc.vector.tensor_copy(rb_sb, ps_rb)
                obuf = wkB.tile([128, BCH], bf16, tag="obuf",
                                name=f"obuf{h}_{j}")
                nc.vector.tensor_mul(obuf, pend["ps_o"], rb_sb)
                nc.sync.dma_start(out=a2a_in[h][j, :, :], in_=obuf)

            pending = None
            for h in range(HPC):
                for b in range(B):
                    for tc2 in range(NBCH):
                        t0 = b * T + tc2 * BCH
                        pt = ptp.tile([128, NST, BCH], fmm, tag="pt",
                                      name=f"pt{h}_{b}_{tc2}")
                        cur = {"h": h, "b": b, "j": b * NBCH + tc2,
                               "pt": pt, "ps_o": None, "tree_step": 0}
                        for sg in range(NST // 4):
                            ps_s = psS.tile([128, 4, BCH], f32, tag="pss",
                                            name=f"pss{h}_{b}_{tc2}_{sg}")
                            for si in range(4):
                                st = sg * 4 + si
                                nc.tensor.matmul(
                                    ps_s[:, si, :],
                                    kt_sb[h][:,
                                             b * T + st * 128:
                                             b * T + (st + 1) * 128],
                                    qt_sb[h][:, t0:t0 + BCH],
                                    start=True, stop=True)
                            nc.scalar.activation(
                                out=pt[:, sg * 4:(sg + 1) * 4, :],
                                in_=ps_s, func=Act.Exp, scale=SCALE)
                            if pending is not None:
                                emit_av(pending,
                                        [sg * 4 + k for k in range(4)])
                                if sg >= 1 and pending["tree_step"] < 5:
                                    emit_tree_step(pending,
                                                   pending["tree_step"])
                                    pending["tree_step"] += 1
                                    if sg == 3 and pending["tree_step"] < 4:
                                        emit_tree_step(
                                            pending, pending["tree_step"])
                                        pending["tree_step"] += 1
                            last = (b == B - 1 and tc2 == NBCH - 1)
                            if last and sg > 0:
                                # drain chunk: AV rides one group behind exp
                                emit_av(cur,
                                        [sg * 4 - 4 + k for k in range(4)])
                        if pending is not None:
                            emit_epilogue(pending)
                        pending = cur
                # drain before this head's collective
                emit_av(pending, [NST - 4 + k for k in range(4)])
                emit_epilogue(pending)
                pending = None
                # per-head AllToAll; head 0's overlaps head 1's compute
                cc_inst = nc.gpsimd.collective_compute(
                    "AllToAll", mybir.AluOpType.bypass,
                    replica_groups=[list(range(NCORES))],
                    ins=[a2a_in[h].opt()], outs=[a2a_out[h].opt()])
                if h == 0:
                    # xr0 staging can run under head 1's attention
                    # (gpsimd queue: its wait on A2A#1 blocks nothing)
                    nc.gpsimd.dma_start(
                        out=xr0_sb,
                        in_=a2a_out[0].rearrange("c p r -> p c r"))
        mides.close()

        # ---------------- Phase C: output projection ----------------
        pcB = es.enter_context(tc.tile_pool(name="pcB", bufs=1))
        outp = es.enter_context(tc.tile_pool(name="outp", bufs=3))
        psC = es.enter_context(tc.tile_pool(name="psC", bufs=8, space="PSUM"))
        xr1_sb = pcB.tile([128, NCORES, ROWS], bf16)
        xr_sb = [xr0_sb, xr1_sb]
        wpiece = [pcB.tile([128, NFT, 256], bf16, name=f"wp{p}")
                  for p in range(8)]
        from concourse.bass import _add_dep_helper
        for p in range(8):
            eng = nc.sync if p % 2 == 0 else nc.scalar
            dma = eng.dma_start(out=wpiece[p], in_=wout[:, p, :, :])
            _add_dep_helper(dma.ins, cc_inst.ins, sync=False,
                            reason="keep wout loads out of the A2A#2 wait set")
        nc.scalar.dma_start(out=xr1_sb,
                            in_=a2a_out[1].rearrange("c p r -> p c r"))
        partial_sb = pcB.tile([128, NFT, ROWS], f32)
        for hh in range(HPC):
            for fti in range(NFT):
                ps_c = psC.tile([128, ROWS], f32, tag="psc",
                                name=f"psc{hh}_{fti}")
                for c in range(NCORES):
                    nc.tensor.matmul(
                        ps_c,
                        wpiece[fti // 2][:, HPC * c + hh,
                                         (fti % 2) * 128:
                                         (fti % 2) * 128 + 128],
                        xr_sb[hh][:, c, :],
                        start=(c == 0), stop=(c == NCORES - 1))
                if hh == 0:
                    nc.scalar.activation(out=partial_sb[:, fti, :],
                                         in_=ps_c, func=Act.Identity,
                                         bias=bout_sb[:, fti:fti + 1])
                else:
                    out_sb = outp.tile([128, ROWS], f32, tag="ob")
                    nc.vector.tensor_add(out_sb, ps_c,
                                         partial_sb[:, fti, :])
                    nc.sync.dma_start(
                        out=outT[fti * 128:(fti + 1) * 128, :],
                        in_=out_sb)

    nc.compile()
    return nc


def get_nc():
    key = ("nc", MM_DTYPE)
    if key not in _CACHE:
        _CACHE[key] = _build()
    return _CACHE[key]


def make_in_maps(query, W_qkv, b_qkv, W_out, b_out):
    query = np.asarray(query, dtype=np.float32)
    W_qkv = np.asarray(W_qkv, dtype=np.float32)
    b_qkv = np.asarray(b_qkv, dtype=np.float32)
    import ml_dtypes
    W_out_bf = (np.asarray(W_out, dtype=np.float32)
                .astype(ml_dtypes.bfloat16)
                .reshape(NFT, 128, 8, 256).transpose(1, 2, 0, 3))
    W_out_bf = np.ascontiguousarray(W_out_bf)  # [128, 8, NFT, 256]
    b_out = np.ascontiguousarray(np.asarray(b_out, dtype=np.float32))

    x = query.reshape(NT, D)
    xT = x.T.reshape(NFT, 128, NT // 256, 256).transpose(1, 2, 0, 3)
    xT = np.ascontiguousarray(xT)          # [128, NACH, NFT, ACH]

    in_maps = []
    for c in range(NCORES):
        h0, h1 = HPC * c, HPC * c + 1
        cols, bcols = [], []
        for blk in range(3):  # q, k, v blocks of W_qkv
            for h in (h0, h1):
                sl = slice(blk * D + h * DH, blk * D + (h + 1) * DH)
                cols.append(W_qkv[:, sl])
                bcols.append(b_qkv[sl])
        wq = (np.stack(cols, axis=0)            # [6, D, DH]
              .reshape(6, NFT, 128, DH).transpose(2, 0, 1, 3))
        in_maps.append({
            "xT": xT,
            "wqkv": np.ascontiguousarray(wq),   # [128, 6, NFT, DH]
            "bqkv": np.ascontiguousarray(np.stack(bcols, axis=0)),
            "wout": W_out_bf,
            "bout": b_out,
        })
    return in_maps


def kernel(query, key, value, W_qkv, b_qkv, W_out, b_out):
    from concourse.bass_utils import run_bass_kernel_spmd

    nc = get_nc()
    in_maps = make_in_maps(query, W_qkv, b_qkv, W_out, b_out)
    res = run_bass_kernel_spmd(nc, in_maps, list(range(NCORES)))
    out = np.empty((NT, D), dtype=np.float32)
    for c in range(NCORES):
        out[c * ROWS:(c + 1) * ROWS, :] = res.results[c]["outT"].T
    return out.reshape(B, T, D)


# revision 30
# speedup vs baseline: 1.3711x; 1.0631x over previous
"""Multi-head attention (B=2, T=2048, D=2048, H=16) on 8 trn2 NeuronCores.

Sharding: tensor-parallel over heads. Core c owns heads {2c, 2c+1}:
  - QKV projection for its 2 heads (Q^T/K^T in [dh, t] layout, V in [t, dh]).
  - Attention per (head, batch), computed as S^T = K^T.T Q^T so softmax probs
    land in [s, t] layout and feed the AV matmul directly (no transposes).
    Softmax skips max-subtraction (scores are O(15) here; exp stays well
    inside fp32 range). The denominator is accumulated on DVE, partition-
    reduced by a ones-matvec on PE, reciprocated at [1, t] and broadcast back
    over partitions with a rank-1 PE matmul.
  - Two per-head AllToAlls redistribute attention outputs from head-sharded
    [dh, t] blocks to row-sharded x^T [D, 512] per core; the first overlaps
    with the second head's attention. Each core then computes 512 rows of the
    output projection with the full W_out (streamed).
Host assembles the 8 row-shards (each returned transposed) into [B, T, D].

All main matmuls run in float32r (single-pass PE fp32). Inputs are declared
float32r in DRAM so plain HWDGE DMAs (no casting) feed SBUF.
"""

import numpy as np

D = 2048
H = 16
DH = 128
B = 2
T = 2048
NT = B * T            # 4096 flattened rows
NCORES = 8
HPC = H // NCORES     # heads per core = 2
ROWS = NT // NCORES   # output rows per core = 512
NFT = D // 128        # 16 feature tiles
SCALE = float(np.sqrt(np.float32(DH)) / np.sqrt(np.float32(D)))  # 0.25

MM_DTYPE = "f32r"

_CACHE = {}


def _build():
    from contextlib import ExitStack

    import concourse.bass as bass  # noqa: F401
    import concourse.mybir as mybir
    import concourse.tile as tile
    from concourse import bacc

    f32 = mybir.dt.float32
    fmm = mybir.dt.float32r if MM_DTYPE == "f32r" else f32
    bf16 = mybir.dt.bfloat16
    Act = mybir.ActivationFunctionType

    nc = bacc.Bacc("TRN2", target_bir_lowering=False, debug=False,
                   num_devices=NCORES)

    ACH = 256                # phase-A t-chunk width
    NACH = NT // ACH         # 16 chunks

    # inputs are pre-arranged on the host into SBUF-image layouts so every
    # load is a fully contiguous per-partition DMA
    xT = nc.dram_tensor("xT", [128, NACH, NFT, ACH], fmm,
                        kind="ExternalInput")
    wqkv = nc.dram_tensor("wqkv", [128, 6, NFT, DH], fmm,
                          kind="ExternalInput")
    bqkv = nc.dram_tensor("bqkv", [6, DH], f32, kind="ExternalInput")
    wout = nc.dram_tensor("wout", [128, 8, NFT, 256], bf16,
                          kind="ExternalInput")
    bout = nc.dram_tensor("bout", [D], f32, kind="ExternalInput")
    outT = nc.dram_tensor("outT", [D, ROWS], f32, kind="ExternalOutput")
    BCH = 512                # phase-B t-chunk width
    NBCH = T // BCH          # 4 chunks per (head, batch)
    NST = T // 128           # 16 s-tiles per batch

    with tile.TileContext(nc) as tc, ExitStack() as es:
        persist = es.enter_context(tc.tile_pool(name="persist", bufs=1))
        dram = es.enter_context(tc.tile_pool(name="dram", bufs=1,
                                             space="DRAM"))
        a2a_in = [dram.tile([NCORES, DH, ROWS], bf16, name=f"a2a_in{h}")
                  for h in range(HPC)]
        a2a_out = [dram.tile([NCORES, DH, ROWS], bf16, name=f"a2a_out{h}")
                   for h in range(HPC)]

        bqkv_sb = persist.tile([128, 6], f32)
        bv_sb = persist.tile([128, HPC * DH], f32)
        bout_sb = persist.tile([128, NFT], f32)
        ones_sb = persist.tile([128, 128], f32)
        ones_r = persist.tile([128, 128], fmm)

        nc.sync.dma_start(out=bqkv_sb, in_=bqkv[:, :].transpose([1, 0]))
        nc.sync.dma_start(out=bv_sb,
                          in_=bqkv[4:6, :].flatten().unsqueeze(0)
                          .to_broadcast([128, HPC * DH]))
        nc.sync.dma_start(out=bout_sb,
                          in_=bout.rearrange("(n p) -> p n", p=128))
        nc.vector.memset(ones_sb, 1.0)
        nc.vector.tensor_copy(ones_r, ones_sb)

        xr0_sb = persist.tile([128, NCORES, ROWS], bf16)
        mides = ExitStack()
        mid = mides.enter_context(tc.tile_pool(name="mid", bufs=1))
        qt_sb = [mid.tile([128, NT], fmm, name=f"qt{h}") for h in range(HPC)]
        kt_sb = [mid.tile([128, NT], fmm, name=f"kt{h}") for h in range(HPC)]
        v_sb = [mid.tile([128, B, NST, DH], fmm, name=f"v{h}")
                for h in range(HPC)]

        # ---------------- Phase A: QKV projection ----------------
        with tc.tile_pool(name="phaseA", bufs=1) as pa, \
             tc.tile_pool(name="xtp", bufs=2) as xtp, \
             tc.tile_pool(name="psA", bufs=2, space="PSUM") as psA:
            wqkv_sb = pa.tile([128, 6, NFT, DH], fmm)
            # slot-major loads: the first Q chain only needs slot 0 (1MB);
            # xt0 goes first on the scalar queue in quarters so the first
            # accumulation chain starts as soon as slot 0 + quarter 0 land
            # slot 0 (first Q chain) split in halves on sync
            for hlf in range(2):
                nc.sync.dma_start(
                    out=wqkv_sb[:, 0, 8 * hlf:8 * hlf + 8, :],
                    in_=wqkv[:, 0, 8 * hlf:8 * hlf + 8, :])
            nc.sync.dma_start(out=wqkv_sb[:, 2, :, :], in_=wqkv[:, 2, :, :])
            nc.sync.dma_start(out=wqkv_sb[:, 4, :, :], in_=wqkv[:, 4, :, :])

            for tch in range(NACH):
                t0 = tch * ACH
                xt_tile = xtp.tile([128, NFT, ACH], fmm, tag="xt")
                if tch == 0:
                    for q in range(4):
                        nc.scalar.dma_start(
                            out=xt_tile[:, 4 * q:4 * q + 4, :],
                            in_=xT[:, 0, 4 * q:4 * q + 4, :])
                    for s in (1, 3, 5):
                        nc.scalar.dma_start(out=wqkv_sb[:, s, :, :],
                                            in_=wqkv[:, s, :, :])
                else:
                    nc.gpsimd.dma_start(out=xt_tile, in_=xT[:, tch, :, :])
                for h in range(HPC):
                    ps_q = psA.tile([128, ACH], f32, tag="psq")
                    for ft in range(NFT):
                        nc.tensor.matmul(
                            ps_q,
                            wqkv_sb[:, h, ft, :], xt_tile[:, ft, :],
                            start=(ft == 0), stop=(ft == NFT - 1))
                    nc.scalar.activation(
                        out=qt_sb[h][:, t0:t0 + ACH], in_=ps_q,
                        func=Act.Identity, bias=bqkv_sb[:, h:h + 1])
                    ps_k = psA.tile([128, ACH], f32, tag="psk")
                    for ft in range(NFT):
                        nc.tensor.matmul(
                            ps_k,
                            wqkv_sb[:, 2 + h, ft, :], xt_tile[:, ft, :],
                            start=(ft == 0), stop=(ft == NFT - 1))
                    nc.scalar.activation(
                        out=kt_sb[h][:, t0:t0 + ACH], in_=ps_k,
                        func=Act.Identity, bias=bqkv_sb[:, 2 + h:3 + h])
                for st in range(ACH // 128):
                    ps_v = psA.tile([128, HPC * DH], f32, tag="psv")
                    for ft in range(NFT):
                        nc.tensor.matmul(
                            ps_v,
                            xt_tile[:, ft, st * 128:(st + 1) * 128],
                            wqkv_sb[:, 4:6, ft, :],
                            start=(ft == 0), stop=(ft == NFT - 1))
                    g = t0 + st * 128
                    b_idx, st_b = g // T, (g % T) // 128
                    for h in range(HPC):
                        nc.vector.tensor_add(
                            v_sb[h][:, b_idx, st_b, :],
                            ps_v[:, h * DH:(h + 1) * DH],
                            bv_sb[:, h * DH:(h + 1) * DH])

        # ---------------- Phase B: attention (h outer, split A2A) --------
        # Software pipeline: chunk k's AV matmuls interleave with chunk k+1's
        # S matmuls so PE stays busy while ACT works through the exps.
        with tc.tile_pool(name="ptp", bufs=2) as ptp, \
             tc.tile_pool(name="wkB", bufs=2) as wkB, \
             tc.tile_pool(name="psS", bufs=2, space="PSUM") as psS, \
             tc.tile_pool(name="psO", bufs=3, space="PSUM") as psO, \
             tc.tile_pool(name="psX", bufs=1, space="PSUM") as psX:

            def emit_av(pend, st_list):
                h, b, j, pt_p = pend["h"], pend["b"], pend["j"], pend["pt"]
                if pend["ps_o"] is None:
                    ps_o = psO.tile([128, BCH], f32, tag="pso",
                                    name=f"pso{h}_{j}")
                    pend["ps_o"] = ps_o
                for st in st_list:
                    nc.tensor.matmul(
                        pend["ps_o"],
                        v_sb[h][:, b, st, :], pt_p[:, st, :],
                        start=(st == 0), stop=(st == NST - 1))

            def emit_tree_step(pend, step):
                h, j, pt_p = pend["h"], pend["j"], pend["pt"]
                ptf = pt_p.bitcast(f32)
                if step == 0:
                    acc4 = wkB.tile([128, 4, BCH], fmm, tag="acc4", bufs=1,
                                    name=f"acc4_{h}_{j}")
                    pend["acc4"] = acc4
                acc4 = pend["acc4"]
                acc4f = acc4.bitcast(f32)
                if step == 0:
                    nc.vector.tensor_add(acc4, ptf[:, 0:4, :],
                                         ptf[:, 4:8, :])
                elif step == 1:
                    nc.vector.tensor_add(acc4, acc4f, ptf[:, 8:12, :])
                elif step == 2:
                    nc.vector.tensor_add(acc4, acc4f, ptf[:, 12:16, :])
                elif step == 3:
                    nc.vector.tensor_add(acc4[:, 0:2, :], acc4f[:, 0:2, :],
                                         acc4f[:, 2:4, :])
                else:
                    nc.vector.tensor_add(acc4[:, 0, :], acc4f[:, 0, :],
                                         acc4f[:, 1, :])

            def emit_epilogue(pend):
                h, b, j, pt_p = pend["h"], pend["b"], pend["j"], pend["pt"]
                for step in range(pend["tree_step"], 5):
                    emit_tree_step(pend, step)
                acc4 = pend["acc4"]
                ps_den = psX.tile([128, BCH], f32, tag="psx",
                                  name=f"psden{h}_{j}")
                nc.tensor.matmul(ps_den[0:1, :], ones_r[:, 0:1],
                                 acc4.bitcast(fmm)[:, 0, :],
                                 start=True, stop=True)
                den_sb = wkB.tile([1, BCH], f32, tag="densb",
                                  name=f"den{h}_{j}")
                nc.vector.reciprocal_approx_fast(den_sb[0:1, :],
                                                 ps_den[0:1, :])
                den_r = wkB.tile([1, BCH], fmm, tag="denr",
                                 name=f"denr{h}_{j}")
                nc.vector.tensor_copy(den_r[0:1, :], den_sb[0:1, :])
                ps_rb = psX.tile([128, BCH], f32, tag="psx",
                                 name=f"psrb{h}_{j}")
                nc.tensor.matmul(ps_rb, ones_r[0:1, :], den_r[0:1, :],
                                 start=True, stop=True)
                rb_sb = wkB.tile([128, BCH], f32, tag="rb",
                                 name=f"rb{h}_{j}")
                nc.vector.tensor_copy(rb_sb, ps_rb)
                obuf = wkB.tile([128, BCH], bf16, tag="obuf",
                                name=f"obuf{h}_{j}")
                nc.vector.tensor_mul(obuf, pend["ps_o"], rb_sb)
                nc.sync.dma_start(out=a2a_in[h][j, :, :], in_=obuf)

            pending = None
            for h in range(HPC):
                for b in range(B):
                    for tc2 in range(NBCH):
                        t0 = b * T + tc2 * BCH
                        pt = ptp.tile([128, NST, BCH], fmm, tag="pt",
                                      name=f"pt{h}_{b}_{tc2}")
                        cur = {"h": h, "b": b, "j": b * NBCH + tc2,
                               "pt": pt, "ps_o": None, "tree_step": 0}
                        for sg in range(NST // 2):
                            ps_s = psS.tile([128, 2, BCH], f32, tag="pss",
                                            name=f"pss{h}_{b}_{tc2}_{sg}")
                            for si in range(2):
                                st = sg * 2 + si
                                nc.tensor.matmul(
                                    ps_s[:, si, :],
                                    kt_sb[h][:,
                                             b * T + st * 128:
                                             b * T + (st + 1) * 128],
                                    qt_sb[h][:, t0:t0 + BCH],
                                    start=True, stop=True)
                            nc.scalar.activation(
                                out=pt[:, sg * 2:(sg + 1) * 2, :],
                                in_=ps_s, func=Act.Exp, scale=SCALE)
                            if pending is not None:
                                emit_av(pending, [sg * 2, sg * 2 + 1])
                                if sg >= 3 and pending["tree_step"] < 5:
                                    emit_tree_step(pending,
                                                   pending["tree_step"])
                                    pending["tree_step"] += 1
                            last = (b == B - 1 and tc2 == NBCH - 1)
                            if last and sg > 0:
                                # drain chunk: AV rides one batch behind exp
                                emit_av(cur, [sg * 2 - 2, sg * 2 - 1])
                        if pending is not None:
                            emit_epilogue(pending)
                        pending = cur
                # drain before this head's collective
                emit_av(pending, [NST - 2, NST - 1])
                emit_epilogue(pending)
                pending = None
                # per-head AllToAll; head 0's overlaps head 1's compute
                cc_inst = nc.gpsimd.collective_compute(
                    "AllToAll", mybir.AluOpType.bypass,
                    replica_groups=[list(range(NCORES))],
                    ins=[a2a_in[h].opt()], outs=[a2a_out[h].opt()])
                if h == 0:
                    # xr0 staging can run under head 1's attention
                    # (gpsimd queue: its wait on A2A#1 blocks nothing)
                    nc.gpsimd.dma_start(
                        out=xr0_sb,
                        in_=a2a_out[0].rearrange("c p r -> p c r"))
        mides.close()

        # ---------------- Phase C: output projection ----------------
        pcB = es.enter_context(tc.tile_pool(name="pcB", bufs=1))
        outp = es.enter_context(tc.tile_pool(name="outp", bufs=3))
        psC = es.enter_context(tc.tile_pool(name="psC", bufs=8, space="PSUM"))
        xr1_sb = pcB.tile([128, NCORES, ROWS], bf16)
        xr_sb = [xr0_sb, xr1_sb]
        wpiece = [pcB.tile([128, NFT, 256], bf16, name=f"wp{p}")
                  for p in range(8)]
        from concourse.bass import _add_dep_helper
        for p in range(8):
            eng = nc.sync if p % 2 == 0 else nc.scalar
            dma = eng.dma_start(out=wpiece[p], in_=wout[:, p, :, :])
            _add_dep_helper(dma.ins, cc_inst.ins, sync=False,
                            reason="keep wout loads out of the A2A#2 wait set")
        nc.scalar.dma_start(out=xr1_sb,
                            in_=a2a_out[1].rearrange("c p r -> p c r"))
        partial_sb = pcB.tile([128, NFT, ROWS], f32)
        for hh in range(HPC):
            for fti in range(NFT):
                ps_c = psC.tile([128, ROWS], f32, tag="psc",
                                name=f"psc{hh}_{fti}")
                for c in range(NCORES):
                    nc.tensor.matmul(
                        ps_c,
                        wpiece[fti // 2][:, HPC * c + hh,
                                         (fti % 2) * 128:
                                         (fti % 2) * 128 + 128],
                        xr_sb[hh][:, c, :],
                        start=(c == 0), stop=(c == NCORES - 1))
                if hh == 0:
                    nc.scalar.activation(out=partial_sb[:, fti, :],
                                         in_=ps_c, func=Act.Identity,
                                         bias=bout_sb[:, fti:fti + 1])
                else:
                    out_sb = outp.tile([128, ROWS], f32, tag="ob")
                    nc.vector.tensor_add(out_sb, ps_c,
                                         partial_sb[:, fti, :])
                    nc.sync.dma_start(
                        out=outT[fti * 128:(fti + 1) * 128, :],
                        in_=out_sb)

    nc.compile()
    return nc


def get_nc():
    key = ("nc", MM_DTYPE)
    if key not in _CACHE:
        _CACHE[key] = _build()
    return _CACHE[key]


def make_in_maps(query, W_qkv, b_qkv, W_out, b_out):
    query = np.asarray(query, dtype=np.float32)
    W_qkv = np.asarray(W_qkv, dtype=np.float32)
    b_qkv = np.asarray(b_qkv, dtype=np.float32)
    import ml_dtypes
    W_out_bf = (np.asarray(W_out, dtype=np.float32)
                .astype(ml_dtypes.bfloat16)
                .reshape(NFT, 128, 8, 256).transpose(1, 2, 0, 3))
    W_out_bf = np.ascontiguousarray(W_out_bf)  # [128, 8, NFT, 256]
    b_out = np.ascontiguousarray(np.asarray(b_out, dtype=np.float32))

    x = query.reshape(NT, D)
    xT = x.T.reshape(NFT, 128, NT // 256, 256).transpose(1, 2, 0, 3)
    xT = np.ascontiguousarray(xT)          # [128, NACH, NFT, ACH]

    in_maps = []
    for c in range(NCORES):
        h0, h1 = HPC * c, HPC * c + 1
        cols, bcols = [], []
        for blk in range(3):  # q, k, v blocks of W_qkv
            for h in (h0, h1):
                sl = slice(blk * D + h * DH, blk * D + (h + 1) * DH)
                cols.append(W_qkv[:, sl])
                bcols.append(b_qkv[sl])
        wq = (np.stack(cols, axis=0)            # [6, D, DH]
              .reshape(6, NFT, 128, DH).transpose(2, 0, 1, 3))
        in_maps.append({
            "xT": xT,
            "wqkv": np.ascontiguousarray(wq),   # [128, 6, NFT, DH]
            "bqkv": np.ascontiguousarray(np.stack(bcols, axis=0)),
            "wout": W_out_bf,
            "bout": b_out,
        })
    return in_maps


def kernel(query, key, value, W_qkv, b_qkv, W_out, b_out):
    from concourse.bass_utils import run_bass_kernel_spmd

    nc = get_nc()
    in_maps = make_in_maps(query, W_qkv, b_qkv, W_out, b_out)
    res = run_bass_kernel_spmd(nc, in_maps, list(range(NCORES)))
    out = np.empty((NT, D), dtype=np.float32)
    for c in range(NCORES):
        out[c * ROWS:(c + 1) * ROWS, :] = res.results[c]["outT"].T
    return out.reshape(B, T, D)
